# revision 11
# baseline (speedup 1.0000x reference)
"""Trainium2 Bass kernel for a full attention block (QKV proj + RMSNorm + RoPE +
softmax attention + output proj), batch-data-parallel across 8 NeuronCores.

Shapes (hardcoded): x (8, 1024, 1024), H=16 heads, hd=64.
Each core processes one batch element; weights are replicated.

Per-core dataflow (all matmuls in float32r = full-speed, ~tf32 precision):
  phase 1: lhsT = x.T blocks (stationary), rhs = w_qkv.T n-slices (streamed).
           psum (128 l, 512 d).  RMS factors from psum; fq folded into the
           per-head q evacuation; fk/8 saved and folded into the exp scale.
           RoPE applied per (n-slice, l-block) with norm weights folded into
           host-side tables.
  phase 2: PE transposes q_r, k_r -> head-major qT, kT (d on partitions).
  phase 3: per head: S.T = kT_blk.T @ qT (j on partitions), exp on ACT
           (psum -> sbuf f32r, scale = fk/8 per partition), PV with a
           ones-augmented V so softmax sums come out as psum row 64;
           normalization via K=1 broadcast matmul + evac multiply.
  phase 4: y = out.T.T @ w_proj.T with out.T blocks stationary.
"""
import numpy as np

import concourse.bass as bass
from concourse import bacc
import concourse.mybir as mybir
import concourse.tile as tile
from concourse.bass_utils import run_bass_kernel_spmd
from concourse.masks import make_identity

F32 = mybir.dt.float32
F32R = mybir.dt.float32r
AF = mybir.ActivationFunctionType
ALU = mybir.AluOpType

B, L, C, H, HD = 8, 1024, 1024, 16, 64
EPS = 1e-6
NLB = L // 128   # 8 l-blocks
NCB = C // 128   # 8 c-blocks
NJB = L // 128   # 8 j-blocks
N_CORES = 8

_nc_cache = None
_last_results = None  # BassKernelResults of the most recent run (for test.py)


def _bcast(ap2d, reps):
    """(128, w) AP -> (128, reps, w) stride-0 broadcast view."""
    return bass.AP(tensor=ap2d.tensor, offset=ap2d.offset,
                   ap=[ap2d.ap[0], [0, reps], ap2d.ap[1]])


def _sub(ap3d, lo, w):
    """(128, reps, 64) bcast view -> free-dim slice [lo:lo+w]."""
    return bass.AP(tensor=ap3d.tensor, offset=ap3d.offset + lo,
                   ap=[ap3d.ap[0], ap3d.ap[1], [1, w]])


def build_nc():
    nc = bacc.Bacc("TRN2", target_bir_lowering=False)

    xT = nc.declare_dram_parameter("xT", [C, L], F32, isOutput=False)
    wq = nc.declare_dram_parameter("wq", [C, 3 * C], F32, isOutput=False)
    wp = nc.declare_dram_parameter("wp", [C, C], F32, isOutput=False)
    # RoPE tables with rms-norm weights folded in (host-prepared)
    cq = nc.declare_dram_parameter("cq", [L, HD], F32, isOutput=False)
    sq = nc.declare_dram_parameter("sq", [L, HD], F32, isOutput=False)
    ck = nc.declare_dram_parameter("ck", [L, HD], F32, isOutput=False)
    sk = nc.declare_dram_parameter("sk", [L, HD], F32, isOutput=False)
    y = nc.declare_dram_parameter("y", [L, C], F32, isOutput=True)

    def tab_view(t):
        # (L, 64) DRAM -> SBUF (128, 8, 64): element (p, lc, j) = t[128*lc + p, j]
        return bass.AP(tensor=t, offset=0,
                       ap=[[HD, 128], [128 * HD, NLB], [1, HD]])

    with tile.TileContext(nc) as tc:
        with tc.tile_pool(name="persist", bufs=1) as persist:
            # --- persistent tiles ---
            cq_sb = persist.tile([128, NLB, HD], F32)
            sq_sb = persist.tile([128, NLB, HD], F32)
            ck_sb = persist.tile([128, NLB, HD], F32)
            sk_sb = persist.tile([128, NLB, HD], F32)
            nc.sync.dma_start(out=cq_sb, in_=tab_view(cq))
            nc.sync.dma_start(out=sq_sb, in_=tab_view(sq))
            nc.sync.dma_start(out=ck_sb, in_=tab_view(ck))
            nc.sync.dma_start(out=sk_sb, in_=tab_view(sk))

            fk_all = persist.tile([128, NLB, H], F32)          # fk/8 per (j, head)
            vb = persist.tile([128, NLB, H, HD + 1], F32R)     # V blocks + ones col
            ident_f = persist.tile([128, 128], F32)
            make_identity(nc, ident_f)
            ident = persist.tile([128, 128], F32R)
            nc.vector.tensor_copy(ident, ident_f)
            ones_f = persist.tile([1, HD], F32)
            nc.vector.memset(ones_f, 1.0)
            ones_r = persist.tile([1, HD], F32R)
            nc.vector.tensor_copy(ones_r, ones_f)
            eps_q = persist.tile([128, 1], F32)
            nc.vector.memset(eps_q, EPS)
            eps_k = persist.tile([128, 1], F32)
            nc.vector.memset(eps_k, HD * EPS)

            ones128 = persist.tile([128, 1], F32)
            nc.vector.memset(ones128, 1.0)
            nc.vector.tensor_copy(
                bass.AP(tensor=vb.tensor, offset=vb.offset + HD,
                        ap=[vb.ap[0], vb.ap[1], vb.ap[2], [1, 1]]),
                bass.AP(tensor=ones128.tensor, offset=ones128.offset,
                        ap=[ones128.ap[0], [0, NLB], [0, H], [1, 1]]))

            with tc.tile_pool(name="pbig", bufs=3) as pbig:
                xr = pbig.tile([128, NCB, L], F32R, tag="big", name="xr")
                qr = pbig.tile([128, NLB, C], F32R, tag="big", name="qr")
                kr = pbig.tile([128, NLB, C], F32R, tag="big", name="kr")

                # ---------------- phase 1: qkv + rms + rope ----------------
                with nc.named_scope("qkv"), \
                     tc.tile_pool(name="p1", bufs=1) as p1, \
                     tc.tile_pool(name="ps1", bufs=4, space="PSUM") as ps1:

                    # element (p, cc, l) = xT[128*cc + p, l]
                    nc.sync.dma_start(out=xr, in_=bass.AP(
                        tensor=xT, offset=0,
                        ap=[[L, 128], [128 * L, NCB], [1, L]]).bitcast(F32R))

                    def rope8(st, dst3, cos_sb, sin_sb, lb):
                        # st (128, 512) f32 = 8 heads; dst3 (128, 8, 64) f32r slice
                        s3 = st.rearrange("p (h d) -> p h d", d=HD)
                        cw = _bcast(cos_sb[:, lb, :], 8)
                        sw = _bcast(sin_sb[:, lb, :], 8)
                        a_t = p1.tile([128, 8, HD], F32, tag="ropeA", bufs=2)
                        nc.vector.tensor_mul(a_t, s3, cw)
                        b_t = p1.tile([128, 8, HD], F32, tag="ropeB", bufs=2)
                        nc.vector.tensor_mul(b_t[:, :, 0:32], s3[:, :, 32:64],
                                             _sub(sw, 0, 32))
                        nc.vector.tensor_mul(b_t[:, :, 32:64], s3[:, :, 0:32],
                                             _sub(sw, 32, 32))
                        nc.vector.tensor_add(dst3, a_t, b_t)

                    for n in range(6):
                        wqn = p1.tile([128, NCB, 512], F32R, tag="wqn", bufs=2)
                        # element (p, cc, j) = wq[128*cc + p, 512*n + j]
                        nc.sync.dma_start(out=wqn, in_=bass.AP(
                            tensor=wq, offset=512 * n,
                            ap=[[3 * C, 128], [128 * 3 * C, NCB], [1, 512]]
                        ).bitcast(F32R))

                        for lb in range(NLB):
                            ps = ps1.tile([128, 512], F32, tag="ps")
                            for cb in range(NCB):
                                nc.tensor.matmul(
                                    ps, lhsT=xr[:, cb, 128 * lb:128 * (lb + 1)],
                                    rhs=wqn[:, cb, :],
                                    start=(cb == 0), stop=(cb == NCB - 1))
                            if n < 4:  # q (n<2) or k (n<4): rms stats
                                sqt = p1.tile([128, 512], F32, tag="sqt", bufs=2)
                                nc.scalar.activation(sqt, ps, AF.Square)
                                sst = p1.tile([128, 8], F32, tag="sst", bufs=2)
                                nc.vector.tensor_reduce(
                                    sst, sqt.rearrange("p (h d) -> p h d", d=HD),
                                    axis=mybir.AxisListType.X, op=ALU.add)
                            if n < 2:      # q heads 8n .. 8n+7
                                rt = p1.tile([128, 8], F32, tag="rt", bufs=2)
                                nc.scalar.activation(rt, sst, AF.Sqrt,
                                                     scale=1.0 / HD, bias=eps_q)
                                fqv = p1.tile([128, 8], F32, tag="fqv", bufs=2)
                                nc.vector.reciprocal(fqv, rt)
                                st = p1.tile([128, 512], F32, tag="stage", bufs=2)
                                for h8 in range(8):
                                    nc.vector.tensor_scalar_mul(
                                        st[:, 64 * h8:64 * (h8 + 1)],
                                        ps[:, 64 * h8:64 * (h8 + 1)],
                                        fqv[:, h8:h8 + 1])
                                rope8(st, qr[:, lb, 512 * n:512 * (n + 1)]
                                      .rearrange("p (h d) -> p h d", d=HD),
                                      cq_sb, sq_sb, lb)
                            elif n < 4:    # k heads 8(n-2)..: fk/8 = 1/sqrt(ss+64eps)
                                rt = p1.tile([128, 8], F32, tag="rt", bufs=2)
                                nc.scalar.activation(rt, sst, AF.Sqrt,
                                                     scale=1.0, bias=eps_k)
                                with nc.allow_low_precision("fk f32"):
                                    nc.vector.reciprocal(
                                        fk_all[:, lb, 8 * (n - 2):8 * (n - 1)], rt)
                                st = p1.tile([128, 512], F32, tag="stage", bufs=2)
                                nc.vector.tensor_copy(st, ps)
                                rope8(st, kr[:, lb, 512 * (n - 2):512 * (n - 1)]
                                      .rearrange("p (h d) -> p h d", d=HD),
                                      ck_sb, sk_sb, lb)
                            else:          # v heads 8(n-4) ..
                                nc.vector.tensor_copy(
                                    vb[:, lb, 8 * (n - 4):8 * (n - 3), 0:HD],
                                    ps.rearrange("p (h d) -> p h d", d=HD))

                # ---------------- phase 2: transposes ----------------
                qT = pbig.tile([128, NCB, L], F32R, tag="big", name="qT")
                kT = pbig.tile([128, NCB, L], F32R, tag="big", name="kT")
                with nc.named_scope("transp"), \
                     tc.tile_pool(name="ps2", bufs=4, space="PSUM") as ps2:
                    for src, dstT in ((qr, qT), (kr, kT)):
                        for lc in range(NLB):
                            for dc in range(NCB):
                                pt = ps2.tile([128, 128], F32R, tag="pt")
                                nc.tensor.transpose(
                                    pt, src[:, lc, 128 * dc:128 * (dc + 1)], ident)
                                nc.vector.tensor_copy(
                                    dstT[:, dc, 128 * lc:128 * (lc + 1)], pt)

                # ---------------- phase 3: attention ----------------
                outT = pbig.tile([128, NCB, L], F32R, tag="big", name="outT")
                with nc.named_scope("attn"), \
                     tc.tile_pool(name="p3", bufs=1) as p3, \
                     tc.tile_pool(name="ps_s", bufs=2, space="PSUM") as ps_s, \
                     tc.tile_pool(name="ps_o", bufs=2, space="PSUM") as ps_o, \
                     tc.tile_pool(name="ps_f", bufs=1, space="PSUM") as ps_f:
                    for h in range(H):
                        hp = 64 * (h % 2)
                        hc = h // 2
                        pso = ps_o.tile([HD + 1, L], F32, tag="pso")
                        for jb in range(NJB):
                            lhs_k = kT[hp:hp + HD, hc, 128 * jb:128 * (jb + 1)]
                            pt_t = p3.tile([128, L], F32R, tag="ptile", bufs=3)
                            for hf in range(2):
                                ps_st = ps_s.tile([128, 512], F32, tag="ps_st")
                                nc.tensor.matmul(
                                    ps_st, lhsT=lhs_k,
                                    rhs=qT[hp:hp + HD, hc, 512 * hf:512 * (hf + 1)],
                                    start=True, stop=True)
                                nc.scalar.activation(
                                    pt_t[:, 512 * hf:512 * (hf + 1)],
                                    ps_st, AF.Exp,
                                    scale=fk_all[:, jb, h:h + 1])
                            for hf in range(2):
                                nc.tensor.matmul(
                                    pso[:, 512 * hf:512 * (hf + 1)],
                                    lhsT=vb[:, jb, h, :],
                                    rhs=pt_t[:, 512 * hf:512 * (hf + 1)],
                                    start=(jb == 0), stop=(jb == NJB - 1))
                        # normalize: rs=1/sums; F=ones64 x rs; outT_h=pso*F
                        rs = p3.tile([1, L], F32R, tag="rs", bufs=2)
                        with nc.allow_low_precision("recip f32r"):
                            nc.vector.reciprocal(rs, pso[HD:HD + 1, :])
                        psf = ps_f.tile([HD, L], F32, tag="psf")
                        for hf in range(2):
                            nc.tensor.matmul(
                                psf[:, 512 * hf:512 * (hf + 1)],
                                lhsT=ones_r,
                                rhs=rs[:, 512 * hf:512 * (hf + 1)],
                                start=True, stop=True)
                        fsb = p3.tile([HD, L], F32, tag="fsb", bufs=2)
                        nc.scalar.copy(fsb, psf)
                        for hf in range(2):
                            nc.vector.tensor_mul(
                                outT[hp:hp + HD, hc, 512 * hf:512 * (hf + 1)],
                                pso[0:HD, 512 * hf:512 * (hf + 1)],
                                fsb[:, 512 * hf:512 * (hf + 1)])

                # ---------------- phase 4: output projection ----------------
                with nc.named_scope("proj"), \
                     tc.tile_pool(name="p4", bufs=1) as p4, \
                     tc.tile_pool(name="ps4", bufs=4, space="PSUM") as ps4:
                    for hf in range(2):
                        wpn = p4.tile([128, NCB, 512], F32R, tag="wpn", bufs=2)
                        nc.sync.dma_start(out=wpn, in_=bass.AP(
                            tensor=wp, offset=512 * hf,
                            ap=[[C, 128], [128 * C, NCB], [1, 512]]
                        ).bitcast(F32R))
                        for lb in range(NLB):
                            psy = ps4.tile([128, 512], F32, tag="psy")
                            for cb in range(NCB):
                                nc.tensor.matmul(
                                    psy,
                                    lhsT=outT[:, cb, 128 * lb:128 * (lb + 1)],
                                    rhs=wpn[:, cb, :],
                                    start=(cb == 0), stop=(cb == NCB - 1))
                            ysb = p4.tile([128, 512], F32, tag="ysb", bufs=3)
                            nc.vector.tensor_copy(ysb, psy)
                            nc.sync.dma_start(
                                out=y[128 * lb:128 * (lb + 1),
                                      512 * hf:512 * (hf + 1)],
                                in_=ysb)

    nc.compile()
    return nc


def _get_nc():
    global _nc_cache
    if _nc_cache is None:
        _nc_cache = build_nc()
    return _nc_cache


def _host_prep(x, cos, sin, w_qkv, w_proj, q_norm_w, k_norm_w):
    x = np.asarray(x, dtype=np.float32)
    cos = np.asarray(cos, dtype=np.float32)
    sin = np.asarray(sin, dtype=np.float32)
    w_qkv = np.asarray(w_qkv, dtype=np.float32)
    w_proj = np.asarray(w_proj, dtype=np.float32)
    q_norm_w = np.asarray(q_norm_w, dtype=np.float32)
    k_norm_w = np.asarray(k_norm_w, dtype=np.float32)

    wqT = np.ascontiguousarray(w_qkv.T)            # (C, 3C)
    wpT = np.ascontiguousarray(w_proj.T)           # (C, C)

    def fold(w):
        # cosW[l,d] = cos[l,d]*w[d]
        # sinW[l,d<32] = -sin[l,d]*w[d+32]; sinW[l,d>=32] = sin[l,d]*w[d-32]
        cosW = cos * w[None, :]
        w_rot = np.concatenate([w[32:], w[:32]])
        sinW = (sin * w_rot[None, :]).copy()
        sinW[:, :32] *= -1.0
        return np.ascontiguousarray(cosW), np.ascontiguousarray(sinW)

    cqt, sqt = fold(q_norm_w)
    ckt, skt = fold(k_norm_w)

    in_maps = []
    for b in range(N_CORES):
        in_maps.append({
            "xT": np.ascontiguousarray(x[b].T),
            "wq": wqT, "wp": wpT,
            "cq": cqt, "sq": sqt, "ck": ckt, "sk": skt,
        })
    return in_maps


def kernel(x, cos, sin, w_qkv, w_proj, q_norm_w, k_norm_w, _trace=False):
    global _last_results
    nc = _get_nc()
    in_maps = _host_prep(x, cos, sin, w_qkv, w_proj, q_norm_w, k_norm_w)
    r = run_bass_kernel_spmd(nc, in_maps, list(range(N_CORES)), trace=_trace)
    _last_results = r
    return np.stack([r.results[b]["y"] for b in range(N_CORES)], axis=0)


# revision 14
# speedup vs baseline: 1.3522x; 1.3522x over previous
"""Trainium2 Bass kernel for a full attention block (QKV proj + RMSNorm + RoPE +
softmax attention + output proj), batch-data-parallel across 8 NeuronCores.

Shapes (hardcoded): x (8, 1024, 1024), H=16 heads, hd=64.
Each core processes one batch element; weights are replicated.

Per-core dataflow (all matmuls in float32r = full-speed, ~tf32 precision):
  phase 1: lhsT = x.T blocks (stationary), rhs = w_qkv.T n-slices (streamed).
           psum (128 l, 512 d).  RMS factors from psum; fq folded into the
           per-head q evacuation; fk/8 saved and folded into the exp scale.
           RoPE applied per (n-slice, l-block) with norm weights folded into
           host-side tables.
  phase 2: PE transposes q_r, k_r -> head-major qT, kT (d on partitions).
  phase 3: per head: S.T = kT_blk.T @ qT (j on partitions), exp on ACT
           (psum -> sbuf f32r, scale = fk/8 per partition), PV with a
           ones-augmented V so softmax sums come out as psum row 64;
           normalization via K=1 broadcast matmul + evac multiply.
  phase 4: y = out.T.T @ w_proj.T with out.T blocks stationary.
"""
import numpy as np

import concourse.bass as bass
from concourse import bacc
import concourse.mybir as mybir
import concourse.tile as tile
from concourse.bass_utils import run_bass_kernel_spmd
from concourse.masks import make_identity

F32 = mybir.dt.float32
F32R = mybir.dt.float32r
AF = mybir.ActivationFunctionType
ALU = mybir.AluOpType

B, L, C, H, HD = 8, 1024, 1024, 16, 64
EPS = 1e-6
NLB = L // 128   # 8 l-blocks
NCB = C // 128   # 8 c-blocks
NJB = L // 128   # 8 j-blocks
N_CORES = 8

_nc_cache = None
_last_results = None  # BassKernelResults of the most recent run (for test.py)


def _bcast(ap2d, reps):
    """(128, w) AP -> (128, reps, w) stride-0 broadcast view."""
    return bass.AP(tensor=ap2d.tensor, offset=ap2d.offset,
                   ap=[ap2d.ap[0], [0, reps], ap2d.ap[1]])


def _sub(ap3d, lo, w):
    """(128, reps, 64) bcast view -> free-dim slice [lo:lo+w]."""
    return bass.AP(tensor=ap3d.tensor, offset=ap3d.offset + lo,
                   ap=[ap3d.ap[0], ap3d.ap[1], [1, w]])


def build_nc():
    nc = bacc.Bacc("TRN2", target_bir_lowering=False)

    xT = nc.declare_dram_parameter("xT", [C, L], F32, isOutput=False)
    wq = nc.declare_dram_parameter("wq", [C, 3 * C], F32, isOutput=False)
    wp = nc.declare_dram_parameter("wp", [C, C], F32, isOutput=False)
    # RoPE tables with rms-norm weights folded in (host-prepared)
    cq = nc.declare_dram_parameter("cq", [L, HD], F32, isOutput=False)
    sq = nc.declare_dram_parameter("sq", [L, HD], F32, isOutput=False)
    ck = nc.declare_dram_parameter("ck", [L, HD], F32, isOutput=False)
    sk = nc.declare_dram_parameter("sk", [L, HD], F32, isOutput=False)
    y = nc.declare_dram_parameter("y", [L, C], F32, isOutput=True)

    def tab_view(t):
        # (L, 64) DRAM -> SBUF (128, 8, 64): element (p, lc, j) = t[128*lc + p, j]
        return bass.AP(tensor=t, offset=0,
                       ap=[[HD, 128], [128 * HD, NLB], [1, HD]])

    with tile.TileContext(nc) as tc:
        with tc.tile_pool(name="persist", bufs=1) as persist:
            # --- persistent tiles ---
            cq_sb = persist.tile([128, NLB, HD], F32)
            sq_sb = persist.tile([128, NLB, HD], F32)
            ck_sb = persist.tile([128, NLB, HD], F32)
            sk_sb = persist.tile([128, NLB, HD], F32)
            nc.sync.dma_start(out=cq_sb, in_=tab_view(cq))
            nc.sync.dma_start(out=sq_sb, in_=tab_view(sq))
            nc.sync.dma_start(out=ck_sb, in_=tab_view(ck))
            nc.sync.dma_start(out=sk_sb, in_=tab_view(sk))

            fk_all = persist.tile([128, NLB, H], F32)          # fk/8 per (j, head)
            vb = persist.tile([128, NLB, H, HD + 1], F32R)     # V blocks + ones col
            ident_f = persist.tile([128, 128], F32)
            make_identity(nc, ident_f)
            ident = persist.tile([128, 128], F32R)
            nc.vector.tensor_copy(ident, ident_f)
            ones_f = persist.tile([1, HD], F32)
            nc.vector.memset(ones_f, 1.0)
            ones_r = persist.tile([1, HD], F32R)
            nc.vector.tensor_copy(ones_r, ones_f)
            eps_q = persist.tile([128, 1], F32)
            nc.vector.memset(eps_q, EPS)
            eps_k = persist.tile([128, 1], F32)
            nc.vector.memset(eps_k, HD * EPS)

            ones128 = persist.tile([128, 1], F32)
            nc.vector.memset(ones128, 1.0)
            nc.vector.tensor_copy(
                bass.AP(tensor=vb.tensor, offset=vb.offset + HD,
                        ap=[vb.ap[0], vb.ap[1], vb.ap[2], [1, 1]]),
                bass.AP(tensor=ones128.tensor, offset=ones128.offset,
                        ap=[ones128.ap[0], [0, NLB], [0, H], [1, 1]]))

            with tc.tile_pool(name="pbig", bufs=3) as pbig:
                xr = pbig.tile([128, NCB, L], F32R, tag="big", name="xr")
                qr = pbig.tile([128, NLB, C], F32R, tag="big", name="qr")
                kr = pbig.tile([128, NLB, C], F32R, tag="big", name="kr")

                # ---------------- phase 1: qkv + rms + rope ----------------
                with nc.named_scope("qkv"), \
                     tc.tile_pool(name="p1", bufs=1) as p1, \
                     tc.tile_pool(name="ps1", bufs=4, space="PSUM") as ps1:

                    # element (p, cc, l) = xT[128*cc + p, l]
                    nc.sync.dma_start(out=xr, in_=bass.AP(
                        tensor=xT, offset=0,
                        ap=[[L, 128], [128 * L, NCB], [1, L]]).bitcast(F32R))

                    def rope8(st, dst3, cos_sb, sin_sb, lb):
                        # st (128, 512) f32 = 8 heads; dst3 (128, 8, 64) f32r slice
                        s3 = st.rearrange("p (h d) -> p h d", d=HD)
                        cw = _bcast(cos_sb[:, lb, :], 8)
                        sw = _bcast(sin_sb[:, lb, :], 8)
                        a_t = p1.tile([128, 8, HD], F32, tag="ropeA", bufs=2)
                        nc.vector.tensor_mul(a_t, s3, cw)
                        b_t = p1.tile([128, 8, HD], F32, tag="ropeB", bufs=2)
                        nc.vector.tensor_mul(b_t[:, :, 0:32], s3[:, :, 32:64],
                                             _sub(sw, 0, 32))
                        nc.vector.tensor_mul(b_t[:, :, 32:64], s3[:, :, 0:32],
                                             _sub(sw, 32, 32))
                        nc.vector.tensor_add(dst3, a_t, b_t)

                    for n in range(6):
                        wqn = p1.tile([128, NCB, 512], F32R, tag="wqn", bufs=2)
                        # element (p, cc, j) = wq[128*cc + p, 512*n + j]
                        nc.sync.dma_start(out=wqn, in_=bass.AP(
                            tensor=wq, offset=512 * n,
                            ap=[[3 * C, 128], [128 * 3 * C, NCB], [1, 512]]
                        ).bitcast(F32R))

                        for lb in range(NLB):
                            ps = ps1.tile([128, 512], F32, tag="ps")
                            for cb in range(NCB):
                                nc.tensor.matmul(
                                    ps, lhsT=xr[:, cb, 128 * lb:128 * (lb + 1)],
                                    rhs=wqn[:, cb, :],
                                    start=(cb == 0), stop=(cb == NCB - 1))
                            if n < 4:  # q (n<2) or k (n<4): rms stats
                                sqt = p1.tile([128, 512], F32, tag="sqt", bufs=2)
                                nc.scalar.activation(sqt, ps, AF.Square)
                                sst = p1.tile([128, 8], F32, tag="sst", bufs=2)
                                nc.vector.tensor_reduce(
                                    sst, sqt.rearrange("p (h d) -> p h d", d=HD),
                                    axis=mybir.AxisListType.X, op=ALU.add)
                            if n < 2:      # q heads 8n .. 8n+7
                                rt = p1.tile([128, 8], F32, tag="rt", bufs=2)
                                nc.scalar.activation(rt, sst, AF.Sqrt,
                                                     scale=1.0 / HD, bias=eps_q)
                                fqv = p1.tile([128, 8], F32, tag="fqv", bufs=2)
                                nc.vector.reciprocal_approx_fast(fqv, rt)
                                st = p1.tile([128, 512], F32, tag="stage", bufs=2)
                                for h8 in range(8):
                                    nc.vector.tensor_scalar_mul(
                                        st[:, 64 * h8:64 * (h8 + 1)],
                                        ps[:, 64 * h8:64 * (h8 + 1)],
                                        fqv[:, h8:h8 + 1])
                                rope8(st, qr[:, lb, 512 * n:512 * (n + 1)]
                                      .rearrange("p (h d) -> p h d", d=HD),
                                      cq_sb, sq_sb, lb)
                            elif n < 4:    # k heads 8(n-2)..: fk/8 = 1/sqrt(ss+64eps)
                                rt = p1.tile([128, 8], F32, tag="rt", bufs=2)
                                nc.scalar.activation(rt, sst, AF.Sqrt,
                                                     scale=1.0, bias=eps_k)
                                nc.vector.reciprocal_approx_fast(
                                    fk_all[:, lb, 8 * (n - 2):8 * (n - 1)], rt)
                                st = p1.tile([128, 512], F32, tag="stage", bufs=2)
                                nc.vector.tensor_copy(st, ps)
                                rope8(st, kr[:, lb, 512 * (n - 2):512 * (n - 1)]
                                      .rearrange("p (h d) -> p h d", d=HD),
                                      ck_sb, sk_sb, lb)
                            else:          # v heads 8(n-4) ..
                                nc.vector.tensor_copy(
                                    vb[:, lb, 8 * (n - 4):8 * (n - 3), 0:HD],
                                    ps.rearrange("p (h d) -> p h d", d=HD))

                # ---------------- phase 2: transposes ----------------
                qT = pbig.tile([128, NCB, L], F32R, tag="big", name="qT")
                kT = pbig.tile([128, NCB, L], F32R, tag="big", name="kT")
                with nc.named_scope("transp"), \
                     tc.tile_pool(name="ps2", bufs=4, space="PSUM") as ps2:
                    for src, dstT in ((qr, qT), (kr, kT)):
                        for lc in range(NLB):
                            for dc in range(NCB):
                                pt = ps2.tile([128, 128], F32R, tag="pt")
                                nc.tensor.transpose(
                                    pt, src[:, lc, 128 * dc:128 * (dc + 1)], ident)
                                nc.vector.tensor_copy(
                                    dstT[:, dc, 128 * lc:128 * (lc + 1)], pt)

                # ---------------- phase 3: attention ----------------
                outT = pbig.tile([128, NCB, L], F32R, tag="big", name="outT")
                with nc.named_scope("attn"), \
                     tc.tile_pool(name="p3", bufs=1) as p3, \
                     tc.tile_pool(name="ps_s", bufs=2, space="PSUM") as ps_s, \
                     tc.tile_pool(name="ps_o", bufs=2, space="PSUM") as ps_o:
                    for hpair in range(H // 2):
                        hc = hpair
                        psos = [ps_o.tile([HD + 1, L], F32, tag="pso",
                                          name=f"pso_{hpair}_{i}") for i in range(2)]
                        for jb in range(NJB):
                            pts = []
                            sts = []
                            for i in range(2):
                                hp = 64 * i
                                ps_st = ps_s.tile([128, L], F32, tag="ps_st",
                                                  name=f"st_{hpair}_{jb}_{i}")
                                lhs_k = kT[hp:hp + HD, hc, 128 * jb:128 * (jb + 1)]
                                for hf in range(2):
                                    nc.tensor.matmul(
                                        ps_st[:, 512 * hf:512 * (hf + 1)],
                                        lhsT=lhs_k,
                                        rhs=qT[hp:hp + HD, hc,
                                               512 * hf:512 * (hf + 1)],
                                        start=True, stop=True)
                                sts.append(ps_st)
                            for i in range(2):
                                h = 2 * hpair + i
                                pt_t = p3.tile([128, L], F32R, tag="ptile", bufs=4)
                                nc.scalar.activation(pt_t, sts[i], AF.Exp,
                                                     scale=fk_all[:, jb, h:h + 1])
                                pts.append(pt_t)
                            for i in range(2):
                                h = 2 * hpair + i
                                for hf in range(2):
                                    nc.tensor.matmul(
                                        psos[i][:, 512 * hf:512 * (hf + 1)],
                                        lhsT=vb[:, jb, h, :],
                                        rhs=pts[i][:, 512 * hf:512 * (hf + 1)],
                                        start=(jb == 0), stop=(jb == NJB - 1))
                        # normalize: rs = 1/sums (approx, ~2e-5); bcast on gpsimd
                        for i in range(2):
                            hp = 64 * i
                            pso = psos[i]
                            srow = p3.tile([1, L], F32, tag="srow", bufs=2)
                            nc.vector.tensor_copy(srow, pso[HD:HD + 1, :])
                            rs = p3.tile([1, L], F32, tag="rs", bufs=2)
                            nc.vector.reciprocal_approx_fast(rs, srow)
                            fsb = p3.tile([HD, L], F32, tag="fsb", bufs=2)
                            nc.gpsimd.partition_broadcast(fsb, rs)
                            for hf in range(2):
                                nc.vector.tensor_mul(
                                    outT[hp:hp + HD, hc, 512 * hf:512 * (hf + 1)],
                                    pso[0:HD, 512 * hf:512 * (hf + 1)],
                                    fsb[:, 512 * hf:512 * (hf + 1)])

                # ---------------- phase 4: output projection ----------------
                with nc.named_scope("proj"), \
                     tc.tile_pool(name="p4", bufs=1) as p4, \
                     tc.tile_pool(name="ps4", bufs=4, space="PSUM") as ps4:
                    for hf in range(2):
                        wpn = p4.tile([128, NCB, 512], F32R, tag="wpn", bufs=2)
                        nc.sync.dma_start(out=wpn, in_=bass.AP(
                            tensor=wp, offset=512 * hf,
                            ap=[[C, 128], [128 * C, NCB], [1, 512]]
                        ).bitcast(F32R))
                        for lb in range(NLB):
                            psy = ps4.tile([128, 512], F32, tag="psy")
                            for cb in range(NCB):
                                nc.tensor.matmul(
                                    psy,
                                    lhsT=outT[:, cb, 128 * lb:128 * (lb + 1)],
                                    rhs=wpn[:, cb, :],
                                    start=(cb == 0), stop=(cb == NCB - 1))
                            ysb = p4.tile([128, 512], F32, tag="ysb", bufs=3)
                            nc.vector.tensor_copy(ysb, psy)
                            nc.sync.dma_start(
                                out=y[128 * lb:128 * (lb + 1),
                                      512 * hf:512 * (hf + 1)],
                                in_=ysb)

    nc.compile()
    return nc


def _get_nc():
    global _nc_cache
    if _nc_cache is None:
        _nc_cache = build_nc()
    return _nc_cache


def _host_prep(x, cos, sin, w_qkv, w_proj, q_norm_w, k_norm_w):
    x = np.asarray(x, dtype=np.float32)
    cos = np.asarray(cos, dtype=np.float32)
    sin = np.asarray(sin, dtype=np.float32)
    w_qkv = np.asarray(w_qkv, dtype=np.float32)
    w_proj = np.asarray(w_proj, dtype=np.float32)
    q_norm_w = np.asarray(q_norm_w, dtype=np.float32)
    k_norm_w = np.asarray(k_norm_w, dtype=np.float32)

    wqT = np.ascontiguousarray(w_qkv.T)            # (C, 3C)
    wpT = np.ascontiguousarray(w_proj.T)           # (C, C)

    def fold(w):
        # cosW[l,d] = cos[l,d]*w[d]
        # sinW[l,d<32] = -sin[l,d]*w[d+32]; sinW[l,d>=32] = sin[l,d]*w[d-32]
        cosW = cos * w[None, :]
        w_rot = np.concatenate([w[32:], w[:32]])
        sinW = (sin * w_rot[None, :]).copy()
        sinW[:, :32] *= -1.0
        return np.ascontiguousarray(cosW), np.ascontiguousarray(sinW)

    cqt, sqt = fold(q_norm_w)
    ckt, skt = fold(k_norm_w)

    in_maps = []
    for b in range(N_CORES):
        in_maps.append({
            "xT": np.ascontiguousarray(x[b].T),
            "wq": wqT, "wp": wpT,
            "cq": cqt, "sq": sqt, "ck": ckt, "sk": skt,
        })
    return in_maps


def kernel(x, cos, sin, w_qkv, w_proj, q_norm_w, k_norm_w, _trace=False):
    global _last_results
    nc = _get_nc()
    in_maps = _host_prep(x, cos, sin, w_qkv, w_proj, q_norm_w, k_norm_w)
    r = run_bass_kernel_spmd(nc, in_maps, list(range(N_CORES)), trace=_trace)
    _last_results = r
    return np.stack([r.results[b]["y"] for b in range(N_CORES)], axis=0)


# revision 16
# speedup vs baseline: 1.4085x; 1.0416x over previous
"""Trainium2 Bass kernel for a full attention block (QKV proj + RMSNorm + RoPE +
softmax attention + output proj), batch-data-parallel across 8 NeuronCores.

Shapes (hardcoded): x (8, 1024, 1024), H=16 heads, hd=64.
Each core processes one batch element; weights are replicated.

Per-core dataflow (all matmuls in float32r = full-speed, ~tf32 precision):
  phase 1: lhsT = x.T blocks (stationary), rhs = w_qkv.T n-slices (streamed).
           psum (128 l, 512 d).  RMS factors from psum; fq folded into the
           per-head q evacuation; fk/8 saved and folded into the exp scale.
           RoPE applied per (n-slice, l-block) with norm weights folded into
           host-side tables.
  phase 2: PE transposes q_r, k_r -> head-major qT, kT (d on partitions).
  phase 3: per head: S.T = kT_blk.T @ qT (j on partitions), exp on ACT
           (psum -> sbuf f32r, scale = fk/8 per partition), PV with a
           ones-augmented V so softmax sums come out as psum row 64;
           normalization via K=1 broadcast matmul + evac multiply.
  phase 4: y = out.T.T @ w_proj.T with out.T blocks stationary.
"""
import numpy as np

import concourse.bass as bass
from concourse import bacc
import concourse.mybir as mybir
import concourse.tile as tile
from concourse.bass_utils import run_bass_kernel_spmd
from concourse.masks import make_identity

F32 = mybir.dt.float32
F32R = mybir.dt.float32r
AF = mybir.ActivationFunctionType
ALU = mybir.AluOpType

B, L, C, H, HD = 8, 1024, 1024, 16, 64
EPS = 1e-6
NLB = L // 128   # 8 l-blocks
NCB = C // 128   # 8 c-blocks
NJB = L // 128   # 8 j-blocks
N_CORES = 8

_nc_cache = None
_last_results = None  # BassKernelResults of the most recent run (for test.py)


def _bcast(ap2d, reps):
    """(128, w) AP -> (128, reps, w) stride-0 broadcast view."""
    return bass.AP(tensor=ap2d.tensor, offset=ap2d.offset,
                   ap=[ap2d.ap[0], [0, reps], ap2d.ap[1]])


def _sub(ap3d, lo, w):
    """(128, reps, 64) bcast view -> free-dim slice [lo:lo+w]."""
    return bass.AP(tensor=ap3d.tensor, offset=ap3d.offset + lo,
                   ap=[ap3d.ap[0], ap3d.ap[1], [1, w]])


def build_nc():
    nc = bacc.Bacc("TRN2", target_bir_lowering=False)

    xT = nc.declare_dram_parameter("xT", [C, L], F32, isOutput=False)
    wq = nc.declare_dram_parameter("wq", [C, 3 * C], F32, isOutput=False)
    wp = nc.declare_dram_parameter("wp", [C, C], F32, isOutput=False)
    # RoPE tables with rms-norm weights folded in (host-prepared)
    cq = nc.declare_dram_parameter("cq", [L, HD], F32, isOutput=False)
    sq = nc.declare_dram_parameter("sq", [L, HD], F32, isOutput=False)
    ck = nc.declare_dram_parameter("ck", [L, HD], F32, isOutput=False)
    sk = nc.declare_dram_parameter("sk", [L, HD], F32, isOutput=False)
    y = nc.declare_dram_parameter("y", [L, C], F32, isOutput=True)

    def tab_view(t):
        # (L, 64) DRAM -> SBUF (128, 8, 64): element (p, lc, j) = t[128*lc + p, j]
        return bass.AP(tensor=t, offset=0,
                       ap=[[HD, 128], [128 * HD, NLB], [1, HD]])

    with tile.TileContext(nc) as tc:
        with tc.tile_pool(name="persist", bufs=1) as persist:
            # --- persistent tiles ---
            cq_sb = persist.tile([128, NLB, HD], F32)
            sq_sb = persist.tile([128, NLB, HD], F32)
            ck_sb = persist.tile([128, NLB, HD], F32)
            sk_sb = persist.tile([128, NLB, HD], F32)
            nc.sync.dma_start(out=cq_sb, in_=tab_view(cq))
            nc.sync.dma_start(out=sq_sb, in_=tab_view(sq))
            nc.sync.dma_start(out=ck_sb, in_=tab_view(ck))
            nc.sync.dma_start(out=sk_sb, in_=tab_view(sk))

            fk_all = persist.tile([128, NLB, H], F32)          # fk/8 per (j, head)
            vb = persist.tile([128, NLB, H, HD + 1], F32R)     # V blocks + ones col
            ident_f = persist.tile([128, 128], F32)
            make_identity(nc, ident_f)
            ident = persist.tile([128, 128], F32R)
            nc.vector.tensor_copy(ident, ident_f)
            ones_f = persist.tile([1, HD], F32)
            nc.vector.memset(ones_f, 1.0)
            ones_r = persist.tile([1, HD], F32R)
            nc.vector.tensor_copy(ones_r, ones_f)
            eps_q = persist.tile([128, 1], F32)
            nc.vector.memset(eps_q, EPS)
            eps_k = persist.tile([128, 1], F32)
            nc.vector.memset(eps_k, HD * EPS)

            ones128 = persist.tile([128, 1], F32)
            nc.vector.memset(ones128, 1.0)
            nc.vector.tensor_copy(
                bass.AP(tensor=vb.tensor, offset=vb.offset + HD,
                        ap=[vb.ap[0], vb.ap[1], vb.ap[2], [1, 1]]),
                bass.AP(tensor=ones128.tensor, offset=ones128.offset,
                        ap=[ones128.ap[0], [0, NLB], [0, H], [1, 1]]))

            with tc.tile_pool(name="pbig", bufs=3) as pbig:
                xr = pbig.tile([128, NCB, L], F32R, tag="big", name="xr")
                qr = pbig.tile([128, NLB, C], F32R, tag="big", name="qr")
                kr = pbig.tile([128, NLB, C], F32R, tag="big", name="kr")

                # ---------------- phase 1: qkv + rms + rope ----------------
                with nc.named_scope("qkv"), \
                     tc.tile_pool(name="p1", bufs=1) as p1, \
                     tc.tile_pool(name="ps1", bufs=4, space="PSUM") as ps1:

                    # element (p, cc, l) = xT[128*cc + p, l]
                    nc.sync.dma_start(out=xr, in_=bass.AP(
                        tensor=xT, offset=0,
                        ap=[[L, 128], [128 * L, NCB], [1, L]]).bitcast(F32R))

                    def rope8(st, dst3, cos_sb, sin_sb, lb):
                        # st (128, 512) f32 = 8 heads; dst3 (128, 8, 64) f32r slice
                        s3 = st.rearrange("p (h d) -> p h d", d=HD)
                        cw = _bcast(cos_sb[:, lb, :], 8)
                        sw = _bcast(sin_sb[:, lb, :], 8)
                        a_t = p1.tile([128, 8, HD], F32, tag="ropeA", bufs=2)
                        nc.vector.tensor_mul(a_t, s3, cw)
                        b_t = p1.tile([128, 8, HD], F32, tag="ropeB", bufs=2)
                        nc.vector.tensor_mul(b_t[:, :, 0:32], s3[:, :, 32:64],
                                             _sub(sw, 0, 32))
                        nc.vector.tensor_mul(b_t[:, :, 32:64], s3[:, :, 0:32],
                                             _sub(sw, 32, 32))
                        nc.vector.tensor_add(dst3, a_t, b_t)

                    for n in (0, 2, 4, 1, 3, 5):
                        wqn = p1.tile([128, NCB, 512], F32R, tag="wqn", bufs=2)
                        # element (p, cc, j) = wq[128*cc + p, 512*n + j]
                        nc.sync.dma_start(out=wqn, in_=bass.AP(
                            tensor=wq, offset=512 * n,
                            ap=[[3 * C, 128], [128 * 3 * C, NCB], [1, 512]]
                        ).bitcast(F32R))

                        for lb in range(NLB):
                            ps = ps1.tile([128, 512], F32, tag="ps")
                            for cb in range(NCB):
                                nc.tensor.matmul(
                                    ps, lhsT=xr[:, cb, 128 * lb:128 * (lb + 1)],
                                    rhs=wqn[:, cb, :],
                                    start=(cb == 0), stop=(cb == NCB - 1))
                            if n < 4:  # q (n<2) or k (n<4): rms stats
                                sqt = p1.tile([128, 512], F32, tag="sqt", bufs=2)
                                nc.scalar.activation(sqt, ps, AF.Square)
                                sst = p1.tile([128, 8], F32, tag="sst", bufs=2)
                                nc.vector.tensor_reduce(
                                    sst, sqt.rearrange("p (h d) -> p h d", d=HD),
                                    axis=mybir.AxisListType.X, op=ALU.add)
                            if n < 2:      # q heads 8n .. 8n+7
                                rt = p1.tile([128, 8], F32, tag="rt", bufs=2)
                                nc.scalar.activation(rt, sst, AF.Sqrt,
                                                     scale=1.0 / HD, bias=eps_q)
                                fqv = p1.tile([128, 8], F32, tag="fqv", bufs=2)
                                nc.vector.reciprocal_approx_fast(fqv, rt)
                                st = p1.tile([128, 512], F32, tag="stage", bufs=2)
                                for h8 in range(8):
                                    nc.vector.tensor_scalar_mul(
                                        st[:, 64 * h8:64 * (h8 + 1)],
                                        ps[:, 64 * h8:64 * (h8 + 1)],
                                        fqv[:, h8:h8 + 1])
                                rope8(st, qr[:, lb, 512 * n:512 * (n + 1)]
                                      .rearrange("p (h d) -> p h d", d=HD),
                                      cq_sb, sq_sb, lb)
                            elif n < 4:    # k heads 8(n-2)..: fk/8 = 1/sqrt(ss+64eps)
                                rt = p1.tile([128, 8], F32, tag="rt", bufs=2)
                                nc.scalar.activation(rt, sst, AF.Sqrt,
                                                     scale=1.0, bias=eps_k)
                                nc.vector.reciprocal_approx_fast(
                                    fk_all[:, lb, 8 * (n - 2):8 * (n - 1)], rt)
                                st = p1.tile([128, 512], F32, tag="stage", bufs=2)
                                nc.vector.tensor_copy(st, ps)
                                rope8(st, kr[:, lb, 512 * (n - 2):512 * (n - 1)]
                                      .rearrange("p (h d) -> p h d", d=HD),
                                      ck_sb, sk_sb, lb)
                            else:          # v heads 8(n-4) ..
                                nc.vector.tensor_copy(
                                    vb[:, lb, 8 * (n - 4):8 * (n - 3), 0:HD],
                                    ps.rearrange("p (h d) -> p h d", d=HD))

                # ---------------- phase 2: transposes ----------------
                qT = pbig.tile([128, NCB, L], F32R, tag="big", name="qT")
                kT = pbig.tile([128, NCB, L], F32R, tag="big", name="kT")
                with nc.named_scope("transp"), \
                     tc.tile_pool(name="ps2", bufs=4, space="PSUM") as ps2:
                    for src, dstT in ((qr, qT), (kr, kT)):
                        for dc in range(NCB):
                            for lc in range(NLB):
                                pt = ps2.tile([128, 128], F32R, tag="pt")
                                nc.tensor.transpose(
                                    pt, src[:, lc, 128 * dc:128 * (dc + 1)], ident)
                                nc.vector.tensor_copy(
                                    dstT[:, dc, 128 * lc:128 * (lc + 1)], pt)

                # ---------------- phase 3: attention ----------------
                outT = pbig.tile([128, NCB, L], F32R, tag="big", name="outT")
                with nc.named_scope("attn"), \
                     tc.tile_pool(name="p3", bufs=1) as p3, \
                     tc.tile_pool(name="ps_s", bufs=2, space="PSUM") as ps_s, \
                     tc.tile_pool(name="ps_o", bufs=2, space="PSUM") as ps_o:
                    for hpair in range(H // 2):
                        hc = hpair
                        psos = [ps_o.tile([HD + 1, L], F32, tag="pso",
                                          name=f"pso_{hpair}_{i}") for i in range(2)]
                        for jb in range(NJB):
                            pts = []
                            sts = []
                            for i in range(2):
                                hp = 64 * i
                                ps_st = ps_s.tile([128, L], F32, tag="ps_st",
                                                  name=f"st_{hpair}_{jb}_{i}")
                                lhs_k = kT[hp:hp + HD, hc, 128 * jb:128 * (jb + 1)]
                                for hf in range(2):
                                    nc.tensor.matmul(
                                        ps_st[:, 512 * hf:512 * (hf + 1)],
                                        lhsT=lhs_k,
                                        rhs=qT[hp:hp + HD, hc,
                                               512 * hf:512 * (hf + 1)],
                                        start=True, stop=True)
                                sts.append(ps_st)
                            for i in range(2):
                                h = 2 * hpair + i
                                pt_t = p3.tile([128, L], F32R, tag="ptile", bufs=4)
                                nc.scalar.activation(pt_t, sts[i], AF.Exp,
                                                     scale=fk_all[:, jb, h:h + 1])
                                pts.append(pt_t)
                            for i in range(2):
                                h = 2 * hpair + i
                                for hf in range(2):
                                    nc.tensor.matmul(
                                        psos[i][:, 512 * hf:512 * (hf + 1)],
                                        lhsT=vb[:, jb, h, :],
                                        rhs=pts[i][:, 512 * hf:512 * (hf + 1)],
                                        start=(jb == 0), stop=(jb == NJB - 1))
                        # normalize: rs = 1/sums (approx, ~2e-5); bcast on gpsimd
                        for i in range(2):
                            hp = 64 * i
                            pso = psos[i]
                            srow = p3.tile([1, L], F32, tag="srow", bufs=2)
                            nc.vector.tensor_copy(srow, pso[HD:HD + 1, :])
                            rs = p3.tile([1, L], F32, tag="rs", bufs=2)
                            nc.vector.reciprocal_approx_fast(rs, srow)
                            fsb = p3.tile([HD, L], F32, tag="fsb", bufs=2)
                            nc.gpsimd.partition_broadcast(fsb, rs)
                            for hf in range(2):
                                nc.vector.tensor_mul(
                                    outT[hp:hp + HD, hc, 512 * hf:512 * (hf + 1)],
                                    pso[0:HD, 512 * hf:512 * (hf + 1)],
                                    fsb[:, 512 * hf:512 * (hf + 1)])

                # ---------------- phase 4: output projection ----------------
                with nc.named_scope("proj"), \
                     tc.tile_pool(name="p4", bufs=1) as p4, \
                     tc.tile_pool(name="ps4", bufs=4, space="PSUM") as ps4:
                    for hf in range(2):
                        wpn = p4.tile([128, NCB, 512], F32R, tag="wpn", bufs=2)
                        nc.sync.dma_start(out=wpn, in_=bass.AP(
                            tensor=wp, offset=512 * hf,
                            ap=[[C, 128], [128 * C, NCB], [1, 512]]
                        ).bitcast(F32R))
                        for lb in range(NLB):
                            psy = ps4.tile([128, 512], F32, tag="psy")
                            for cb in range(NCB):
                                nc.tensor.matmul(
                                    psy,
                                    lhsT=outT[:, cb, 128 * lb:128 * (lb + 1)],
                                    rhs=wpn[:, cb, :],
                                    start=(cb == 0), stop=(cb == NCB - 1))
                            ysb = p4.tile([128, 512], F32, tag="ysb", bufs=3)
                            nc.vector.tensor_copy(ysb, psy)
                            nc.sync.dma_start(
                                out=y[128 * lb:128 * (lb + 1),
                                      512 * hf:512 * (hf + 1)],
                                in_=ysb)

    nc.compile()
    return nc


def _get_nc():
    global _nc_cache
    if _nc_cache is None:
        _nc_cache = build_nc()
    return _nc_cache


def _host_prep(x, cos, sin, w_qkv, w_proj, q_norm_w, k_norm_w):
    x = np.asarray(x, dtype=np.float32)
    cos = np.asarray(cos, dtype=np.float32)
    sin = np.asarray(sin, dtype=np.float32)
    w_qkv = np.asarray(w_qkv, dtype=np.float32)
    w_proj = np.asarray(w_proj, dtype=np.float32)
    q_norm_w = np.asarray(q_norm_w, dtype=np.float32)
    k_norm_w = np.asarray(k_norm_w, dtype=np.float32)

    wqT = np.ascontiguousarray(w_qkv.T)            # (C, 3C)
    wpT = np.ascontiguousarray(w_proj.T)           # (C, C)

    def fold(w):
        # cosW[l,d] = cos[l,d]*w[d]
        # sinW[l,d<32] = -sin[l,d]*w[d+32]; sinW[l,d>=32] = sin[l,d]*w[d-32]
        cosW = cos * w[None, :]
        w_rot = np.concatenate([w[32:], w[:32]])
        sinW = (sin * w_rot[None, :]).copy()
        sinW[:, :32] *= -1.0
        return np.ascontiguousarray(cosW), np.ascontiguousarray(sinW)

    cqt, sqt = fold(q_norm_w)
    ckt, skt = fold(k_norm_w)

    in_maps = []
    for b in range(N_CORES):
        in_maps.append({
            "xT": np.ascontiguousarray(x[b].T),
            "wq": wqT, "wp": wpT,
            "cq": cqt, "sq": sqt, "ck": ckt, "sk": skt,
        })
    return in_maps


def kernel(x, cos, sin, w_qkv, w_proj, q_norm_w, k_norm_w, _trace=False):
    global _last_results
    nc = _get_nc()
    in_maps = _host_prep(x, cos, sin, w_qkv, w_proj, q_norm_w, k_norm_w)
    r = run_bass_kernel_spmd(nc, in_maps, list(range(N_CORES)), trace=_trace)
    _last_results = r
    return np.stack([r.results[b]["y"] for b in range(N_CORES)], axis=0)
